# revision 1
# baseline (speedup 1.0000x reference)
"""Causal self-attention with RoPE on 8 Trainium2 NeuronCores.

Problem (hardcoded): B=4, S=2048, D=1024, H=16 heads, hd=64, fp32.
  qkv = x @ w_qkv.T ; rope(q, k) ; causal softmax(q k^T / sqrt(hd)) @ v ; out @ w_out.T

Sharding: core c -> (batch b = c//2, head-group hg = c%2 of 8 heads).
Each core computes a full [S, D] partial output (its heads' contribution to
the output projection); the host sums the two partials per batch.

On-core layout highlights:
  - Q, K are kept transposed ([hd, S], head pairs stacked on 128 partitions) so
    scores are computed as S^T = K^T-tile.T @ Q^T-chunk with zero transposes.
  - A = exp(S^T * scale) is masked causally in-place with gpsimd.affine_select.
  - V tiles carry an appended ones-column, so the A@V PE accumulation also
    produces softmax denominators for free; normalization happens per-head on
    DVE with a DMA partition-broadcast of the reciprocals.
  - Matmuls run as float32r (full-rate, ~tf32-precision fp32) except A@V (bf16).
"""

import sys

if "/opt/trn_rl_repo" not in sys.path:
    sys.path.insert(0, "/opt/trn_rl_repo")

import numpy as np

B, S, D = 4, 2048, 1024
H, HD = 16, 64
NCORES = 8
ROPE_BASE = 10000.0


class Cfg:
    def __init__(self, S=2048, D=1024, HPC=8, QC=512, SC=512, prec="f32r"):
        assert prec in ("bf16", "f32r")
        self.S, self.D, self.HPC, self.QC, self.SC = S, D, HPC, QC, SC
        self.prec = prec
        self.NP = HPC // 2          # head-pair blocks
        self.ND = D // 128          # contraction d-tiles
        self.NSC = S // SC          # projection s-chunks
        self.NQ = S // QC           # attention q-chunks
        self.NST = S // 128         # k/v s-tiles


def build_nc(cfg: Cfg, use_dma_bcast: bool = False):
    """Build the per-core Bass program (SPMD: same program on all 8 cores)."""
    from contextlib import ExitStack

    import concourse.bass as bass
    from concourse import bacc, mybir, tile

    f32 = mybir.dt.float32
    f32r = mybir.dt.float32r
    bf16 = mybir.dt.bfloat16
    wdt = bf16 if cfg.prec == "bf16" else f32r  # matmul operand dtype
    Exp = mybir.ActivationFunctionType.Exp
    is_ge = mybir.AluOpType.is_ge

    Sz, Dz, HPC, QC, SC = cfg.S, cfg.D, cfg.HPC, cfg.QC, cfg.SC
    NP, ND, NSC, NQ, NST = cfg.NP, cfg.ND, cfg.NSC, cfg.NQ, cfg.NST
    scale = float(HD) ** -0.5

    nc = bacc.Bacc("TRN2", target_bir_lowering=False, debug=False)

    xT = nc.dram_tensor("xt", [Dz, Sz], wdt, kind="ExternalInput").ap()
    wqk = nc.dram_tensor("wqk", [Dz, 2 * HPC * HD], wdt, kind="ExternalInput").ap()
    wv = nc.dram_tensor("wv", [Dz, HPC * HD], wdt, kind="ExternalInput").ap()
    wo = nc.dram_tensor("wo", [HPC * HD, Dz], wdt, kind="ExternalInput").ap()
    cosT = nc.dram_tensor("cost", [128, Sz], f32, kind="ExternalInput").ap()
    sinT = nc.dram_tensor("sint", [128, Sz], f32, kind="ExternalInput").ap()
    out = nc.dram_tensor("out", [Sz, Dz], f32, kind="ExternalOutput").ap()

    with tile.TileContext(nc) as tc, ExitStack() as ctx:
        persist = ctx.enter_context(tc.tile_pool(name="persist", bufs=1))

        qt = [persist.tile([128, Sz], wdt, tag=f"qt{p}", name=f"qt{p}") for p in range(NP)]
        kt = [persist.tile([128, Sz], wdt, tag=f"kt{p}", name=f"kt{p}") for p in range(NP)]
        vt = [
            persist.tile([128, HPC * (HD + 1)], bf16, tag=f"vt{si}", name=f"vt{si}")
            for si in range(NST)
        ]
        wo_sb = [persist.tile([128, Dz], wdt, tag=f"wo{c}", name=f"wo{c}") for c in range(NP)]

        for c in range(NP):
            nc.sync.dma_start(wo_sb[c][:], wo[c * 128 : (c + 1) * 128, :])
        for si in range(NST):
            ones_cols = vt[si].rearrange("p (h c) -> p h c", c=HD + 1)[:, :, HD : HD + 1]
            nc.gpsimd.memset(ones_cols, 1.0)

        # ---------------- Phase 1a: Q/K projection + RoPE ----------------
        with (
            tc.tile_pool(name="p1q", bufs=1) as p1,
            tc.tile_pool(name="p1qw", bufs=2) as p1w,
            tc.tile_pool(name="psum1", bufs=3, space="PSUM") as pp1,
        ):
            wqk_sb = [p1.tile([128, 2 * HPC * HD], wdt, tag=f"wqk{d}", name=f"wqk{d}") for d in range(ND)]
            for d in range(ND):
                nc.sync.dma_start(wqk_sb[d][:], wqk[d * 128 : (d + 1) * 128, :])
            cos_sb = p1.tile([128, Sz], f32, tag="cos")
            sin_sb = p1.tile([128, Sz], f32, tag="sin")
            nc.sync.dma_start(cos_sb[:], cosT)
            nc.sync.dma_start(sin_sb[:], sinT)

            for sc_i in range(NSC):
                xts = []
                for d_i in range(ND):
                    t = p1w.tile([128, SC], wdt, tag=f"x{d_i}", name=f"x{d_i}")
                    nc.sync.dma_start(
                        t[:], xT[d_i * 128 : (d_i + 1) * 128, sc_i * SC : (sc_i + 1) * SC]
                    )
                    xts.append(t)
                for g in range(2 * NP):  # first NP groups = q pairs, then k pairs
                    ps = pp1.tile([128, SC], f32, tag="proj")
                    for d_i in range(ND):
                        nc.tensor.matmul(
                            ps[:],
                            wqk_sb[d_i][:, g * 128 : (g + 1) * 128],
                            xts[d_i][:],
                            start=(d_i == 0),
                            stop=(d_i == ND - 1),
                        )
                    dst = qt[g] if g < NP else kt[g - NP]
                    nc.vector.tensor_copy(dst[:, sc_i * SC : (sc_i + 1) * SC], ps[:])

            # RoPE, in place on each 128-partition (head-pair) block.
            # blk' = blk * cosT + swap(blk) * sinT   (signs baked into sinT)
            for blk in [*qt, *kt]:
                tmp = p1w.tile([128, Sz], wdt, tag="rope", bufs=1)
                for dst_p, src_p in ((0, 32), (32, 0), (64, 96), (96, 64)):
                    nc.sync.dma_start(tmp[dst_p : dst_p + 32, :], blk[src_p : src_p + 32, :])
                nc.vector.tensor_mul(tmp[:], tmp[:], sin_sb[:])
                nc.vector.tensor_mul(blk[:], blk[:], cos_sb[:])
                nc.vector.tensor_add(blk[:], blk[:], tmp[:])

        # ---------------- Phase 1b: V projection ----------------
        with (
            tc.tile_pool(name="p1v", bufs=1) as p2,
            tc.tile_pool(name="p1vw", bufs=2) as p2w,
            tc.tile_pool(name="psum2", bufs=3, space="PSUM") as pp2,
        ):
            wv_sb = [p2.tile([128, HPC * HD], wdt, tag=f"wv{d}", name=f"wv{d}") for d in range(ND)]
            for d in range(ND):
                nc.sync.dma_start(wv_sb[d][:], wv[d * 128 : (d + 1) * 128, :])
            for sc_i in range(NSC):
                xts = []
                for d_i in range(ND):
                    t = p2w.tile([128, SC], wdt, tag=f"x{d_i}", name=f"xv{d_i}")
                    nc.sync.dma_start(
                        t[:], xT[d_i * 128 : (d_i + 1) * 128, sc_i * SC : (sc_i + 1) * SC]
                    )
                    xts.append(t)
                for ss in range(SC // 128):
                    ps = pp2.tile([128, HPC * HD], f32, tag="vproj")
                    for d_i in range(ND):
                        nc.tensor.matmul(
                            ps[:],
                            xts[d_i][:, ss * 128 : (ss + 1) * 128],
                            wv_sb[d_i][:],
                            start=(d_i == 0),
                            stop=(d_i == ND - 1),
                        )
                    si = sc_i * (SC // 128) + ss
                    v_cols = vt[si].rearrange("p (h c) -> p h c", c=HD + 1)[:, :, 0:HD]
                    nc.vector.tensor_copy(v_cols, ps.rearrange("p (h c) -> p h c", c=HD))

        # ---------------- Phase 2: attention + output projection ----------------
        with (
            tc.tile_pool(name="att", bufs=min(cfg.NST, cfg.NQ * (QC // 128)) + 2) as aw,
            tc.tile_pool(name="small", bufs=4) as sm,
            tc.tile_pool(name="ost", bufs=3) as ost,
            tc.tile_pool(name="stp", bufs=2, space="PSUM") as stp,
            tc.tile_pool(name="otp", bufs=3, space="PSUM") as otp,
            tc.tile_pool(name="opp", bufs=1, space="PSUM") as opp,
        ):
            for qi in range(NQ):
                nkt = (qi + 1) * QC // 128  # causal: only k-tiles up to the diagonal
                ot = [
                    sm.tile([128, QC], wdt, tag=f"ot{p}", name=f"ot{p}", bufs=2)
                    for p in range(NP)
                ]
                for pb in range(NP):
                    # Even/odd head score matmuls target disjoint PE row groups
                    # (partitions 0-63 / 64-127) and separate PSUM banks, so
                    # each pair runs concurrently; one exp covers both heads.
                    ats = []
                    for ki in range(nkt):
                        st = stp.tile([128, 2 * QC], f32, tag="st")
                        for ph in range(2):
                            prow = ph * 64
                            nc.tensor.matmul(
                                st[:, ph * QC : (ph + 1) * QC],
                                kt[pb][prow : prow + 64, ki * 128 : (ki + 1) * 128],
                                qt[pb][prow : prow + 64, qi * QC : (qi + 1) * QC],
                                start=True,
                                stop=True,
                            )
                        at = aw.tile([128, 2 * QC], bf16, tag="at")
                        nc.scalar.activation(at[:], st[:], Exp, scale=scale)
                        if (ki + 1) * 128 > qi * QC:  # tile overlaps the diagonal
                            for ph in range(2):
                                nc.gpsimd.affine_select(
                                    out=at[:, ph * QC : (ph + 1) * QC],
                                    in_=at[:, ph * QC : (ph + 1) * QC],
                                    compare_op=is_ge,
                                    fill=0.0,
                                    base=qi * QC - ki * 128,
                                    channel_multiplier=-1,
                                    pattern=[[1, QC]],
                                )
                        ats.append(at)
                    # A @ [V | 1]: accumulated over k-tiles; row HD = denominators
                    for ph in range(2):
                        h = 2 * pb + ph
                        prow = ph * 64
                        otps = otp.tile([HD + 1, QC], f32, tag="otp")
                        for ki in range(nkt):
                            nc.tensor.matmul(
                                otps[:],
                                vt[ki][:, h * (HD + 1) : (h + 1) * (HD + 1)],
                                ats[ki][:, ph * QC : (ph + 1) * QC],
                                start=(ki == 0),
                                stop=(ki == nkt - 1),
                            )
                        # normalize: ot = otps[:HD] / denom, broadcast over partitions
                        rec = sm.tile([1, QC], f32, tag="rec")
                        nc.vector.reciprocal(rec[0:1, :], otps[HD : HD + 1, :])
                        rb = sm.tile([64, QC], f32, tag="rb")
                        nc.gpsimd.partition_broadcast(rb[:], rec[0:1, :])
                        nc.vector.tensor_mul(
                            ot[pb][prow : prow + 64, :],
                            otps[0:HD, :],
                            rb[:],
                        )
                # output projection for this q-chunk
                OD = min(512, Dz)
                for dj in range(Dz // OD):
                    for qs in range(QC // 128):
                        q0 = qi * QC + qs * 128
                        po = opp.tile([128, OD], f32, tag="po")
                        for c in range(NP):
                            nc.tensor.matmul(
                                po[:],
                                ot[c][:, qs * 128 : (qs + 1) * 128],
                                wo_sb[c][:, dj * OD : (dj + 1) * OD],
                                start=(c == 0),
                                stop=(c == NP - 1),
                            )
                        ob = ost.tile([128, OD], f32, tag="ob")
                        nc.vector.tensor_copy(ob[:], po[:])
                        nc.sync.dma_start(out[q0 : q0 + 128, dj * OD : (dj + 1) * OD], ob[:])

    nc.compile()
    return nc


def rope_tables(Sz: int):
    """cosT [128, S] and sign-baked sinT [128, S] in the [hd, s] layout.

    q' = q * cosT + swap(q) * sinT, where swap exchanges partition halves
    (0:32 <-> 32:64) within each 64-row block.
    """
    inv_freq = 1.0 / (ROPE_BASE ** (np.arange(0, HD, 2, dtype=np.float32) / HD))
    t = np.arange(Sz, dtype=np.float32)
    freqs = t[:, None] * inv_freq[None, :]  # [S, 32]
    emb = np.concatenate([freqs, freqs], axis=-1)  # [S, 64]
    cos = np.cos(emb).T.astype(np.float32)  # [64, S]
    sin = np.sin(emb).T.astype(np.float32)  # [64, S]
    sin2 = sin.copy()
    sin2[0:32] = -sin2[0:32]
    cosT = np.concatenate([cos, cos], axis=0)
    sinT = np.concatenate([sin2, sin2], axis=0)
    return np.ascontiguousarray(cosT), np.ascontiguousarray(sinT)


def core_inputs(x, w_qkv, w_out, core: int, prec="bf16"):
    """Host-side prep of one core's input map."""
    import ml_dtypes

    ndt = ml_dtypes.bfloat16 if prec == "bf16" else np.float32
    b, hg = core // 2, core % 2
    Dz = x.shape[2]
    hpc_rows = (H // 2) * HD  # 512 rows per head-group
    r0 = hg * hpc_rows
    wq = w_qkv[r0 : r0 + hpc_rows, :]
    wk = w_qkv[Dz + r0 : Dz + r0 + hpc_rows, :]
    wv_ = w_qkv[2 * Dz + r0 : 2 * Dz + r0 + hpc_rows, :]
    cosT, sinT = rope_tables(x.shape[1])
    return {
        "xt": np.ascontiguousarray(x[b].T).astype(ndt),
        "wqk": np.ascontiguousarray(np.concatenate([wq, wk], axis=0).T).astype(ndt),
        "wv": np.ascontiguousarray(wv_.T).astype(ndt),
        "wo": np.ascontiguousarray(w_out[:, r0 : r0 + hpc_rows].T).astype(ndt),
        "cost": cosT,
        "sint": sinT,
    }


_CACHE = {}


def kernel(x, w_qkv, w_out):
    x = np.asarray(x, dtype=np.float32)
    w_qkv = np.asarray(w_qkv, dtype=np.float32)
    w_out = np.asarray(w_out, dtype=np.float32)
    assert x.shape == (B, S, D) and w_qkv.shape == (3 * D, D) and w_out.shape == (D, D)

    from concourse.bass_utils import run_bass_kernel_spmd

    cfg = Cfg()
    if "nc" not in _CACHE:
        _CACHE["nc"] = build_nc(cfg)
    nc = _CACHE["nc"]

    in_maps = [core_inputs(x, w_qkv, w_out, c, prec=cfg.prec) for c in range(NCORES)]
    res = run_bass_kernel_spmd(nc, in_maps, core_ids=list(range(NCORES)))
    outs = [res.results[c]["out"] for c in range(NCORES)]
    full = np.empty((B, S, D), dtype=np.float32)
    for b in range(B):
        full[b] = outs[2 * b] + outs[2 * b + 1]
    return full



# revision 5
# speedup vs baseline: 1.7642x; 1.7642x over previous
"""Causal self-attention with RoPE on 8 Trainium2 NeuronCores.

Problem (hardcoded): B=4, S=2048, D=1024, H=16 heads, hd=64, fp32.
  qkv = x @ w_qkv.T ; rope(q, k) ; causal softmax(q k^T / sqrt(hd)) @ v ; out @ w_out.T

Sharding: core c -> (batch b = c//2, head-group hg = c%2 of 8 heads).
Each core computes a full [S, D] partial output (its heads' contribution to
the output projection); the host sums the two partials per batch.

Single fused pipeline (all bf16 operands, f32 PSUM accumulation):
  per s-chunk ci (512 rows): project Q/K/V for the chunk, RoPE in place,
  then attention for q-chunk qi=ci over k-tiles 0..(ci+1)*4, interleaved
  with the NEXT chunk's projection matmuls so the PE never drains while
  the Act engine works through the exp() stream.

Attention layout:
  - Q, K transposed ([hd, S], head pairs stacked on 128 partitions); scores
    S^T = K-tile.T @ Q-chunk, exp()ed on Act into bf16 `at` tiles [128k, 2*512q].
  - Causal masking only touches the [128,128] diagonal band of each at tile
    (gpsimd affine_select); fully-masked columns are simply never read.
  - A@V runs transposed: stationary = at band [128k, 128q] (full PE array),
    moving = V-tile with appended ones column [128k, 65] -> PSUM [128q, 65]
    accumulated over k-tiles; col 64 = softmax denominator per q (free with
    the ones trick). Normalize = per-partition reciprocal + tensor_scalar.
  - Normalized O [q, d] bf16 is DMA-XBAR-transposed to [d, q] tiles feeding
    the output projection (contraction over d on partitions).
  - Output projections for qi<3 are deferred and interleaved into the final
    (Act-bound) q-chunk so the PE stays busy while exp() drains.
"""

import sys

if "/opt/trn_rl_repo" not in sys.path:
    sys.path.insert(0, "/opt/trn_rl_repo")

import numpy as np

B, S, D = 4, 2048, 1024
H, HD = 16, 64
NCORES = 8
ROPE_BASE = 10000.0

SC = 512          # s-chunk = q-chunk size
NP = 4            # head-pair blocks per core (8 heads)
ND = D // 128     # contraction d-tiles
NC = S // SC      # chunks
NST = S // 128    # k/v s-tiles


class Cfg:
    def __init__(self):
        self.S, self.D, self.SC = S, D, SC


def build_nc(cfg: Cfg = None):
    """Build the per-core Bass program (SPMD: same program on all 8 cores)."""
    from contextlib import ExitStack

    import concourse.bass as bass
    from concourse import bacc, mybir, tile

    f32 = mybir.dt.float32
    bf16 = mybir.dt.bfloat16
    Exp = mybir.ActivationFunctionType.Exp
    is_ge = mybir.AluOpType.is_ge

    scale = float(HD) ** -0.5
    QC = SC

    nc = bacc.Bacc("TRN2", target_bir_lowering=False, debug=False)

    xT = nc.dram_tensor("xt", [D, S], bf16, kind="ExternalInput").ap()
    wqk = nc.dram_tensor("wqk", [D, 2 * NP * 128], bf16, kind="ExternalInput").ap()
    wv = nc.dram_tensor("wv", [D, NP * 128], bf16, kind="ExternalInput").ap()
    wo = nc.dram_tensor("wo", [NP * 128, D], bf16, kind="ExternalInput").ap()
    cosT = nc.dram_tensor("cost", [128, S], bf16, kind="ExternalInput").ap()
    sinT = nc.dram_tensor("sint", [128, S], bf16, kind="ExternalInput").ap()
    out = nc.dram_tensor("out", [S, D], f32, kind="ExternalOutput").ap()

    with tile.TileContext(nc) as tc, ExitStack() as ctx:
        persist = ctx.enter_context(tc.tile_pool(name="persist", bufs=1))
        xp = ctx.enter_context(tc.tile_pool(name="xp", bufs=2))
        ropep = ctx.enter_context(tc.tile_pool(name="ropep", bufs=3))
        atp = ctx.enter_context(tc.tile_pool(name="atp", bufs=18))
        onp = ctx.enter_context(tc.tile_pool(name="onp", bufs=2))
        otTp = ctx.enter_context(tc.tile_pool(name="otTp", bufs=18))
        obp = ctx.enter_context(tc.tile_pool(name="obp", bufs=3))
        recp = ctx.enter_context(tc.tile_pool(name="recp", bufs=4))
        stp = ctx.enter_context(tc.tile_pool(name="stp", bufs=2, space="PSUM"))
        avp = ctx.enter_context(tc.tile_pool(name="avp", bufs=1, space="PSUM"))
        pop = ctx.enter_context(tc.tile_pool(name="pop", bufs=2, space="PSUM"))

        # ---- persistent tiles ----
        qt = [persist.tile([128, S], bf16, tag=f"qt{p}", name=f"qt{p}") for p in range(NP)]
        kt = [persist.tile([128, S], bf16, tag=f"kt{p}", name=f"kt{p}") for p in range(NP)]
        vt = [
            persist.tile([128, 2 * NP * (HD + 1)], bf16, tag=f"vt{si}", name=f"vt{si}")
            for si in range(NST)
        ]
        wqk_sb = persist.tile([128, ND * 1024], bf16, tag="wqk", name="wqk_sb")
        wv_sb = persist.tile([128, ND * 512], bf16, tag="wv", name="wv_sb")
        wo_sb = persist.tile([128, NP * 1024], bf16, tag="wo", name="wo_sb")
        cos_sb = persist.tile([128, S], bf16, tag="cos", name="cos_sb")
        sin_sb = persist.tile([128, S], bf16, tag="sin", name="sin_sb")

        # ---- prologue DMAs ----
        for d in range(ND):
            nc.sync.dma_start(wqk_sb[:, d * 1024 : (d + 1) * 1024], wqk[d * 128 : (d + 1) * 128, :])
        nc.sync.dma_start(cos_sb[:], cosT)
        nc.sync.dma_start(sin_sb[:], sinT)
        for d in range(ND):
            nc.sync.dma_start(wv_sb[:, d * 512 : (d + 1) * 512], wv[d * 128 : (d + 1) * 128, :])
        for c in range(NP):
            nc.sync.dma_start(wo_sb[:, c * 1024 : (c + 1) * 1024], wo[c * 128 : (c + 1) * 128, :])
        for si in range(NST):
            ones_cols = vt[si].rearrange("p (h c) -> p h c", c=HD + 1)[:, :, HD : HD + 1]
            nc.gpsimd.memset(ones_cols, 1.0)

        # ---- emission helpers ----
        def load_x(ci):
            xts = xp.tile([128, ND * SC], bf16, tag="x", name=f"x{ci}")
            for d in range(ND):
                nc.sync.dma_start(
                    xts[:, d * SC : (d + 1) * SC],
                    xT[d * 128 : (d + 1) * 128, ci * SC : (ci + 1) * SC],
                )
            return xts

        def qk_proj_unit(ci, g, xts):
            """Project q-pair (g<NP) or k-pair (g>=NP) block for chunk ci + RoPE."""
            ck = slice(ci * SC, (ci + 1) * SC)
            ps = pop.tile([128, SC], f32, tag="po", name=f"ps{ci}_{g}")
            for d in range(ND):
                nc.tensor.matmul(
                    ps[:],
                    wqk_sb[:, d * 1024 + g * 128 : d * 1024 + (g + 1) * 128],
                    xts[:, d * SC : (d + 1) * SC],
                    start=(d == 0),
                    stop=(d == ND - 1),
                )
            blk = qt[g] if g < NP else kt[g - NP]
            nc.vector.tensor_copy(blk[:, ck], ps[:])
            tmp = ropep.tile([128, SC], bf16, tag="tmp", name=f"tmp{ci}_{g}")
            for dst_p, src_p in ((0, 32), (32, 0), (64, 96), (96, 64)):
                nc.sync.dma_start(tmp[dst_p : dst_p + 32, :], blk[src_p : src_p + 32, ck])
            nc.vector.tensor_mul(tmp[:], tmp[:], sin_sb[:, ck])
            nc.vector.tensor_mul(blk[:, ck], blk[:, ck], cos_sb[:, ck])
            nc.vector.tensor_add(blk[:, ck], blk[:, ck], tmp[:])

        def v_proj_unit(ci, ss, xts):
            ps = pop.tile([128, NP * 128], f32, tag="po", name=f"vp{ci}_{ss}")
            for d in range(ND):
                nc.tensor.matmul(
                    ps[:],
                    xts[:, d * SC + ss * 128 : d * SC + (ss + 1) * 128],
                    wv_sb[:, d * 512 : (d + 1) * 512],
                    start=(d == 0),
                    stop=(d == ND - 1),
                )
            si = ci * 4 + ss
            v_cols = vt[si].rearrange("p (h c) -> p h c", c=HD + 1)[:, :, 0:HD]
            nc.vector.tensor_copy(v_cols, ps.rearrange("p (h c) -> p h c", c=HD))

        def outproj_unit(qi, u, otT):
            """Output projection for global q-tile tq = 4*qi+u from otT [128d,(dt,q)]."""
            q0 = (qi * 4 + u) * 128
            ob = obp.tile([128, D], f32, tag="ob", name=f"ob{qi}_{u}")
            for dj in range(2):
                po = pop.tile([128, 512], f32, tag="po", name=f"po{qi}_{u}_{dj}")
                for dt in range(NP):
                    nc.tensor.matmul(
                        po[:],
                        otT[:, dt * 128 : (dt + 1) * 128],
                        wo_sb[:, dt * 1024 + dj * 512 : dt * 1024 + dj * 512 + 512],
                        start=(dt == 0),
                        stop=(dt == NP - 1),
                    )
                nc.vector.tensor_copy(ob[:, dj * 512 : (dj + 1) * 512], po[:])
            nc.sync.dma_start(out[q0 : q0 + 128, :], ob[:])

        # ---- prologue: chunk 0 projection ----
        xts_cur = load_x(0)
        x_next = [None]

        # deferred output projections: (qi, u, otT tile)
        deferred = []

        for qi in range(NC):
            nkt = (qi + 1) * 4
            # work units to interleave into this q-chunk's attention:
            units = []
            if qi + 1 < NC:
                xts_nxt = load_x(qi + 1)
                x_next[0] = xts_nxt
                for g in range(2 * NP):
                    units.append(("qk", g, xts_nxt))
                for ss in range(4):
                    units.append(("v", ss, xts_nxt))
            else:
                for (dqi, du, dotT) in deferred:
                    units.append(("op", (dqi, du), dotT))

            unit_i = [0]

            def emit_units(n):
                for _ in range(n):
                    if unit_i[0] >= len(units):
                        return
                    kind, a, b = units[unit_i[0]]
                    unit_i[0] += 1
                    if kind == "qk":
                        qk_proj_unit(qi + 1, a, b)
                    elif kind == "v":
                        v_proj_unit(qi + 1, a, b)
                    else:
                        outproj_unit(a[0], a[1], b)

            if qi == 0:
                for g in range(2 * NP):
                    qk_proj_unit(0, g, xts_cur)
                for ss in range(4):
                    v_proj_unit(0, ss, xts_cur)

            # per-pb spread of filler units across the 4 pb stages
            per_pb = (len(units) + NP - 1) // NP

            o_norm = [
                onp.tile([128, 2 * NP * HD], bf16, tag=f"on{u}", name=f"on{qi}_{u}")
                for u in range(4)
            ]

            for pb in range(NP):
                ats = []
                for ki in range(nkt):
                    st = stp.tile([128, 2 * QC], f32, tag="st", name=f"st{qi}_{pb}_{ki}")
                    for ph in range(2):
                        nc.tensor.matmul(
                            st[:, ph * QC : (ph + 1) * QC],
                            kt[pb][ph * 64 : ph * 64 + 64, ki * 128 : (ki + 1) * 128],
                            qt[pb][ph * 64 : ph * 64 + 64, qi * QC : (qi + 1) * QC],
                            start=True,
                            stop=True,
                        )
                    at = atp.tile([128, 2 * QC], bf16, tag="at", name=f"at{qi}_{pb}_{ki}")
                    nc.scalar.activation(at[:], st[:], Exp, scale=scale)
                    if ki >= 4 * qi:  # diagonal band mask
                        u = ki - 4 * qi
                        for ph in range(2):
                            nc.gpsimd.affine_select(
                                out=at[:, ph * QC + u * 128 : ph * QC + (u + 1) * 128],
                                in_=at[:, ph * QC + u * 128 : ph * QC + (u + 1) * 128],
                                compare_op=is_ge,
                                fill=0.0,
                                base=0,
                                channel_multiplier=-1,
                                pattern=[[1, 128]],
                            )
                    ats.append(at)
                    # spread filler matmuls between score tiles (keeps PE fed
                    # while Act drains the exp queue)
                    target = per_pb * pb + (per_pb * (ki + 1)) // nkt
                    emit_units(target - unit_i[0])

                # A@V transposed: psum [128q, 65] per head, accum over k-tiles
                ava = avp.tile([128, 260], f32, tag="ava", name=f"ava{qi}_{pb}")
                avb = avp.tile([128, 260], f32, tag="avb", name=f"avb{qi}_{pb}")
                for u in range(4):
                    av = ava if u < 2 else avb
                    ul = u % 2
                    nk = 4 * qi + u + 1
                    for ph in range(2):
                        off = ul * 130 + ph * 65
                        for ki in range(nk):
                            nc.tensor.matmul(
                                av[:, off : off + 65],
                                ats[ki][:, ph * QC + u * 128 : ph * QC + (u + 1) * 128],
                                vt[ki][:, (2 * pb + ph) * (HD + 1) : (2 * pb + ph + 1) * (HD + 1)],
                                start=(ki == 0),
                                stop=(ki == nk - 1),
                            )
                # normalize: rec[q] = 1/denominator; o_norm cols = head-major
                for av, ug in ((ava, 0), (avb, 2)):
                    rec = recp.tile([128, 4], f32, tag="rec", name=f"rec{qi}_{pb}_{ug}")
                    nc.vector.reciprocal(
                        rec[:], av.rearrange("p (x c) -> p x c", c=65)[:, :, HD]
                    )
                    for ul in range(2):
                        u = ug + ul
                        for ph in range(2):
                            h = 2 * pb + ph
                            nc.vector.tensor_scalar_mul(
                                o_norm[u][:, h * HD : (h + 1) * HD],
                                av[:, ul * 130 + ph * 65 : ul * 130 + ph * 65 + HD],
                                rec[:, 2 * ul + ph : 2 * ul + ph + 1],
                            )
                emit_units(per_pb * (pb + 1) - unit_i[0])

            # transpose normalized O per q-tile: otT[p, dt*128+j] = o_norm[j, dt*128+p]
            own = []
            for u in range(4):
                otT = otTp.tile([128, NP * 128], bf16, tag="otT", name=f"otT{qi}_{u}")
                nc.sync.dma_start(
                    otT.rearrange("p (dt j) -> p dt j", j=128), o_norm[u], transpose=True
                )
                own.append((qi, u, otT))
            if qi + 1 < NC:
                deferred.extend(own)
            else:
                for (dqi, du, dotT) in own:
                    outproj_unit(dqi, du, dotT)
            emit_units(len(units))  # drain any leftovers

            xts_cur = x_next[0]

    nc.compile()
    return nc


def rope_tables(Sz: int):
    """cosT [128, S] and sign-baked sinT [128, S] in the [hd, s] layout.

    q' = q * cosT + swap(q) * sinT, where swap exchanges partition halves
    (0:32 <-> 32:64) within each 64-row block.
    """
    inv_freq = 1.0 / (ROPE_BASE ** (np.arange(0, HD, 2, dtype=np.float32) / HD))
    t = np.arange(Sz, dtype=np.float32)
    freqs = t[:, None] * inv_freq[None, :]  # [S, 32]
    emb = np.concatenate([freqs, freqs], axis=-1)  # [S, 64]
    cos = np.cos(emb).T.astype(np.float32)  # [64, S]
    sin = np.sin(emb).T.astype(np.float32)  # [64, S]
    sin2 = sin.copy()
    sin2[0:32] = -sin2[0:32]
    cosT = np.concatenate([cos, cos], axis=0)
    sinT = np.concatenate([sin2, sin2], axis=0)
    return np.ascontiguousarray(cosT), np.ascontiguousarray(sinT)


def core_inputs(x, w_qkv, w_out, core: int):
    """Host-side prep of one core's input map (bf16)."""
    import ml_dtypes

    ndt = ml_dtypes.bfloat16
    b, hg = core // 2, core % 2
    Dz = x.shape[2]
    hpc_rows = (H // 2) * HD  # 512 rows per head-group
    r0 = hg * hpc_rows
    wq = w_qkv[r0 : r0 + hpc_rows, :]
    wk = w_qkv[Dz + r0 : Dz + r0 + hpc_rows, :]
    wv_ = w_qkv[2 * Dz + r0 : 2 * Dz + r0 + hpc_rows, :]
    cosT, sinT = rope_tables(x.shape[1])
    return {
        "xt": np.ascontiguousarray(x[b].T).astype(ndt),
        "wqk": np.ascontiguousarray(np.concatenate([wq, wk], axis=0).T).astype(ndt),
        "wv": np.ascontiguousarray(wv_.T).astype(ndt),
        "wo": np.ascontiguousarray(w_out[:, r0 : r0 + hpc_rows].T).astype(ndt),
        "cost": cosT.astype(ndt),
        "sint": sinT.astype(ndt),
    }


_CACHE = {}


def kernel(x, w_qkv, w_out):
    x = np.asarray(x, dtype=np.float32)
    w_qkv = np.asarray(w_qkv, dtype=np.float32)
    w_out = np.asarray(w_out, dtype=np.float32)
    assert x.shape == (B, S, D) and w_qkv.shape == (3 * D, D) and w_out.shape == (D, D)

    from concourse.bass_utils import run_bass_kernel_spmd

    if "nc" not in _CACHE:
        _CACHE["nc"] = build_nc(Cfg())
    nc = _CACHE["nc"]

    in_maps = [core_inputs(x, w_qkv, w_out, c) for c in range(NCORES)]
    res = run_bass_kernel_spmd(nc, in_maps, core_ids=list(range(NCORES)))
    outs = [res.results[c]["out"] for c in range(NCORES)]
    full = np.empty((B, S, D), dtype=np.float32)
    for b in range(B):
        full[b] = outs[2 * b] + outs[2 * b + 1]
    return full


# revision 9
# speedup vs baseline: 1.8430x; 1.0447x over previous
"""Causal self-attention with RoPE on 8 Trainium2 NeuronCores.

Problem (hardcoded): B=4, S=2048, D=1024, H=16 heads, hd=64, fp32.
  qkv = x @ w_qkv.T ; rope(q, k) ; causal softmax(q k^T / sqrt(hd)) @ v ; out @ w_out.T

Sharding: core c -> (batch b = c//2, head-group hg = c%2 of 8 heads).
Each core computes a full [S, D] partial output (its heads' contribution to
the output projection); the host sums the two partials per batch.

Single fused pipeline (all bf16 operands, f32 PSUM accumulation):
  per s-chunk ci (512 rows): project Q/K/V for the chunk, RoPE in place,
  then attention for q-chunk qi=ci over k-tiles 0..(ci+1)*4, interleaved
  with the NEXT chunk's projection matmuls so the PE never drains while
  the Act engine works through the exp() stream.

Attention layout:
  - Q, K transposed ([hd, S], head pairs stacked on 128 partitions); scores
    S^T = K-tile.T @ Q-chunk, exp()ed on Act into bf16 `at` tiles [128k, 2*512q].
  - Causal masking only touches the [128,128] diagonal band of each at tile
    (gpsimd affine_select); fully-masked columns are simply never read.
  - A@V runs transposed: stationary = at band [128k, 128q] (full PE array),
    moving = V-tile with appended ones column [128k, 65] -> PSUM [128q, 65]
    accumulated over k-tiles; col 64 = softmax denominator per q (free with
    the ones trick). Normalize = per-partition reciprocal + tensor_scalar.
  - Normalized O [q, d] bf16 is DMA-XBAR-transposed to [d, q] tiles feeding
    the output projection (contraction over d on partitions).
  - Output projections for qi<3 are deferred and interleaved into the final
    (Act-bound) q-chunk so the PE stays busy while exp() drains.
"""

import sys

if "/opt/trn_rl_repo" not in sys.path:
    sys.path.insert(0, "/opt/trn_rl_repo")

import numpy as np

B, S, D = 4, 2048, 1024
H, HD = 16, 64
NCORES = 8
ROPE_BASE = 10000.0

SC = 512          # s-chunk = q-chunk size
NP = 4            # head-pair blocks per core (8 heads)
ND = D // 128     # contraction d-tiles
NC = S // SC      # chunks
NST = S // 128    # k/v s-tiles


class Cfg:
    def __init__(self):
        self.S, self.D, self.SC = S, D, SC


def build_nc(cfg: Cfg = None):
    """Build the per-core Bass program (SPMD: same program on all 8 cores)."""
    from contextlib import ExitStack

    import concourse.bass as bass
    from concourse import bacc, mybir, tile

    f32 = mybir.dt.float32
    bf16 = mybir.dt.bfloat16
    Exp = mybir.ActivationFunctionType.Exp
    is_ge = mybir.AluOpType.is_ge

    scale = float(HD) ** -0.5
    QC = SC

    nc = bacc.Bacc("TRN2", target_bir_lowering=False, debug=False)

    xT = nc.dram_tensor("xt", [D, S], bf16, kind="ExternalInput").ap()
    wqk = nc.dram_tensor("wqk", [D, 2 * NP * 128], bf16, kind="ExternalInput").ap()
    wv = nc.dram_tensor("wv", [D, NP * 128], bf16, kind="ExternalInput").ap()
    wo = nc.dram_tensor("wo", [NP * 128, D], bf16, kind="ExternalInput").ap()
    cosT = nc.dram_tensor("cost", [128, S], bf16, kind="ExternalInput").ap()
    sinT = nc.dram_tensor("sint", [128, S], bf16, kind="ExternalInput").ap()
    out = nc.dram_tensor("out", [S, D], f32, kind="ExternalOutput").ap()

    with tile.TileContext(nc) as tc, ExitStack() as ctx:
        persist = ctx.enter_context(tc.tile_pool(name="persist", bufs=1))
        xp = ctx.enter_context(tc.tile_pool(name="xp", bufs=3))
        ropep = ctx.enter_context(tc.tile_pool(name="ropep", bufs=3))
        atp = ctx.enter_context(tc.tile_pool(name="atp", bufs=18))
        onp = ctx.enter_context(tc.tile_pool(name="onp", bufs=2))
        otTp = ctx.enter_context(tc.tile_pool(name="otTp", bufs=18))
        obp = ctx.enter_context(tc.tile_pool(name="obp", bufs=3))
        recp = ctx.enter_context(tc.tile_pool(name="recp", bufs=4))
        stp = ctx.enter_context(tc.tile_pool(name="stp", bufs=2, space="PSUM"))
        avp = ctx.enter_context(tc.tile_pool(name="avp", bufs=1, space="PSUM"))
        pop = ctx.enter_context(tc.tile_pool(name="pop", bufs=2, space="PSUM"))

        # ---- persistent tiles ----
        qt = [persist.tile([128, S], bf16, tag=f"qt{p}", name=f"qt{p}") for p in range(NP)]
        kt = [persist.tile([128, S], bf16, tag=f"kt{p}", name=f"kt{p}") for p in range(NP)]
        vt = [
            persist.tile([128, 2 * NP * (HD + 1)], bf16, tag=f"vt{si}", name=f"vt{si}")
            for si in range(NST)
        ]
        wqk_sb = persist.tile([128, ND * 1024], bf16, tag="wqk", name="wqk_sb")
        wv_sb = persist.tile([128, ND * 512], bf16, tag="wv", name="wv_sb")
        wo_sb = persist.tile([128, NP * 1024], bf16, tag="wo", name="wo_sb")
        cos_sb = persist.tile([128, S], bf16, tag="cos", name="cos_sb")
        sin_sb = persist.tile([128, S], bf16, tag="sin", name="sin_sb")

        # ---- emission helpers ----
        def load_x(ci, split=1):
            xts = xp.tile([128, ND * SC], bf16, tag="x", name=f"x{ci}")
            dh = ND // split
            for s0 in range(split):
                nc.scalar.dma_start(
                    xts.rearrange("p (d j) -> p d j", j=SC)[:, s0 * dh : (s0 + 1) * dh, :],
                    xT.rearrange("(d p) s -> p d s", p=128)[
                        :, s0 * dh : (s0 + 1) * dh, ci * SC : (ci + 1) * SC
                    ],
                )
            return xts

        # ---- prologue DMAs (halves first so the first matmul chain can start
        # as early as possible; bulk loads issue on the Act DGE queue, which
        # never head-of-line-blocks behind compute-dependent DMAs) ----
        wqk_v = wqk_sb.rearrange("p (d c) -> p d c", c=1024)
        wqk_src = wqk.rearrange("(d p) c -> p d c", p=128)
        nc.scalar.dma_start(wqk_v[:, 0 : ND // 2, :], wqk_src[:, 0 : ND // 2, :])
        nc.scalar.dma_start(wqk_v[:, ND // 2 : ND, :], wqk_src[:, ND // 2 : ND, :])
        xts_cur0 = load_x(0, split=2)
        nc.scalar.dma_start(cos_sb[:], cosT)
        nc.scalar.dma_start(sin_sb[:], sinT)
        nc.scalar.dma_start(
            wv_sb.rearrange("p (d c) -> p d c", c=512), wv.rearrange("(d p) c -> p d c", p=128)
        )
        nc.scalar.dma_start(
            wo_sb.rearrange("p (d c) -> p d c", c=1024), wo.rearrange("(d p) c -> p d c", p=128)
        )
        for si in range(NST):
            ones_cols = vt[si].rearrange("p (h c) -> p h c", c=HD + 1)[:, :, HD : HD + 1]
            nc.gpsimd.memset(ones_cols, 1.0)

        def qk_proj_unit(ci, g, xts):
            """Project q-pair (g<NP) or k-pair (g>=NP) block for chunk ci + RoPE."""
            ck = slice(ci * SC, (ci + 1) * SC)
            ps = pop.tile([128, SC], f32, tag="po", name=f"ps{ci}_{g}")
            for d in range(ND):
                nc.tensor.matmul(
                    ps[:],
                    wqk_sb[:, d * 1024 + g * 128 : d * 1024 + (g + 1) * 128],
                    xts[:, d * SC : (d + 1) * SC],
                    start=(d == 0),
                    stop=(d == ND - 1),
                )
            blk = qt[g] if g < NP else kt[g - NP]
            nc.vector.tensor_copy(blk[:, ck], ps[:])
            tmp = ropep.tile([128, SC], bf16, tag="tmp", name=f"tmp{ci}_{g}")
            for dst_p, src_p in ((0, 32), (32, 0), (64, 96), (96, 64)):
                nc.sync.dma_start(tmp[dst_p : dst_p + 32, :], blk[src_p : src_p + 32, ck])
            nc.vector.tensor_mul(tmp[:], tmp[:], sin_sb[:, ck])
            nc.vector.tensor_mul(blk[:, ck], blk[:, ck], cos_sb[:, ck])
            nc.vector.tensor_add(blk[:, ck], blk[:, ck], tmp[:])

        def v_proj_unit(ci, ss, xts):
            ps = pop.tile([128, NP * 128], f32, tag="po", name=f"vp{ci}_{ss}")
            for d in range(ND):
                nc.tensor.matmul(
                    ps[:],
                    xts[:, d * SC + ss * 128 : d * SC + (ss + 1) * 128],
                    wv_sb[:, d * 512 : (d + 1) * 512],
                    start=(d == 0),
                    stop=(d == ND - 1),
                )
            si = ci * 4 + ss
            v_cols = vt[si].rearrange("p (h c) -> p h c", c=HD + 1)[:, :, 0:HD]
            nc.vector.tensor_copy(v_cols, ps.rearrange("p (h c) -> p h c", c=HD))

        def outproj_unit(qi, u, otT):
            """Output projection for global q-tile tq = 4*qi+u from otT [128d,(dt,q)]."""
            q0 = (qi * 4 + u) * 128
            ob = obp.tile([128, D], f32, tag="ob", name=f"ob{qi}_{u}")
            for dj in range(2):
                po = pop.tile([128, 512], f32, tag="po", name=f"po{qi}_{u}_{dj}")
                for dt in range(NP):
                    nc.tensor.matmul(
                        po[:],
                        otT[:, dt * 128 : (dt + 1) * 128],
                        wo_sb[:, dt * 1024 + dj * 512 : dt * 1024 + dj * 512 + 512],
                        start=(dt == 0),
                        stop=(dt == NP - 1),
                    )
                nc.vector.tensor_copy(ob[:, dj * 512 : (dj + 1) * 512], po[:])
            nc.sync.dma_start(out[q0 : q0 + 128, :], ob[:])

        # ---- prologue: chunk 0 projection ----
        xts_cur = xts_cur0
        x_next = [None]

        # deferred output projections: (qi, u, otT tile)
        deferred = []

        for qi in range(NC):
            nkt = (qi + 1) * 4
            # work units to interleave into this q-chunk's attention:
            units = []
            if qi + 1 < NC:
                xts_nxt = load_x(qi + 1)
                x_next[0] = xts_nxt
                for g in range(2 * NP):
                    units.append(("qk", g, xts_nxt))
                for ss in range(4):
                    units.append(("v", ss, xts_nxt))
            else:
                for (dqi, du, dotT) in deferred:
                    units.append(("op", (dqi, du), dotT))

            unit_i = [0]

            def emit_units(n):
                for _ in range(n):
                    if unit_i[0] >= len(units):
                        return
                    kind, a, b = units[unit_i[0]]
                    unit_i[0] += 1
                    if kind == "qk":
                        qk_proj_unit(qi + 1, a, b)
                    elif kind == "v":
                        v_proj_unit(qi + 1, a, b)
                    else:
                        outproj_unit(a[0], a[1], b)

            if qi == 0:
                for g in range(2 * NP):
                    qk_proj_unit(0, g, xts_cur)
                for ss in range(4):
                    v_proj_unit(0, ss, xts_cur)

            # per-pb spread of filler units across the 4 pb stages
            per_pb = (len(units) + NP - 1) // NP

            o_norm = [
                onp.tile([128, 2 * NP * HD], bf16, tag=f"on{u}", name=f"on{qi}_{u}")
                for u in range(4)
            ]

            for pb in range(NP):
                ats = []
                for ki in range(nkt):
                    st = stp.tile([128, 2 * QC], f32, tag="st", name=f"st{qi}_{pb}_{ki}")
                    for ph in range(2):
                        nc.tensor.matmul(
                            st[:, ph * QC : (ph + 1) * QC],
                            kt[pb][ph * 64 : ph * 64 + 64, ki * 128 : (ki + 1) * 128],
                            qt[pb][ph * 64 : ph * 64 + 64, qi * QC : (qi + 1) * QC],
                            start=True,
                            stop=True,
                        )
                    at = atp.tile([128, 2 * QC], bf16, tag="at", name=f"at{qi}_{pb}_{ki}")
                    nc.scalar.activation(at[:], st[:], Exp, scale=scale)
                    if ki >= 4 * qi:  # diagonal band mask
                        u = ki - 4 * qi
                        for ph in range(2):
                            nc.gpsimd.affine_select(
                                out=at[:, ph * QC + u * 128 : ph * QC + (u + 1) * 128],
                                in_=at[:, ph * QC + u * 128 : ph * QC + (u + 1) * 128],
                                compare_op=is_ge,
                                fill=0.0,
                                base=0,
                                channel_multiplier=-1,
                                pattern=[[1, 128]],
                            )
                    ats.append(at)
                    # spread filler matmuls between score tiles (keeps PE fed
                    # while Act drains the exp queue)
                    target = per_pb * pb + (per_pb * (ki + 1)) // nkt
                    emit_units(target - unit_i[0])

                # A@V transposed: psum [128q, 65] per head, accum over k-tiles.
                # Normalize each accumulator right after its chains finish so
                # the next pb's chains (bufs=1 WAR) wait as little as possible.
                ava = avp.tile([128, 260], f32, tag="ava", name=f"ava{qi}_{pb}")
                avb = avp.tile([128, 260], f32, tag="avb", name=f"avb{qi}_{pb}")
                for av, ug in ((ava, 0), (avb, 2)):
                    for ul in range(2):
                        u = ug + ul
                        nk = 4 * qi + u + 1
                        for ph in range(2):
                            off = ul * 130 + ph * 65
                            for ki in range(nk):
                                nc.tensor.matmul(
                                    av[:, off : off + 65],
                                    ats[ki][:, ph * QC + u * 128 : ph * QC + (u + 1) * 128],
                                    vt[ki][:, (2 * pb + ph) * (HD + 1) : (2 * pb + ph + 1) * (HD + 1)],
                                    start=(ki == 0),
                                    stop=(ki == nk - 1),
                                )
                    rec = recp.tile([128, 4], f32, tag="rec", name=f"rec{qi}_{pb}_{ug}")
                    nc.vector.reciprocal(
                        rec[:], av.rearrange("p (x c) -> p x c", c=65)[:, :, HD]
                    )
                    for ul in range(2):
                        u = ug + ul
                        for ph in range(2):
                            h = 2 * pb + ph
                            nc.vector.tensor_scalar_mul(
                                o_norm[u][:, h * HD : (h + 1) * HD],
                                av[:, ul * 130 + ph * 65 : ul * 130 + ph * 65 + HD],
                                rec[:, 2 * ul + ph : 2 * ul + ph + 1],
                            )
                emit_units(per_pb * (pb + 1) - unit_i[0])

            # transpose normalized O per q-tile: otT[p, dt*128+j] = o_norm[j, dt*128+p]
            own = []
            for u in range(4):
                otT = otTp.tile([128, NP * 128], bf16, tag="otT", name=f"otT{qi}_{u}")
                nc.sync.dma_start(
                    otT.rearrange("p (dt j) -> p dt j", j=128), o_norm[u], transpose=True
                )
                own.append((qi, u, otT))
            if qi + 1 < NC:
                deferred.extend(own)
            else:
                for (dqi, du, dotT) in own:
                    outproj_unit(dqi, du, dotT)
            emit_units(len(units))  # drain any leftovers

            xts_cur = x_next[0]

    nc.compile()
    return nc


def rope_tables(Sz: int):
    """cosT [128, S] and sign-baked sinT [128, S] in the [hd, s] layout.

    q' = q * cosT + swap(q) * sinT, where swap exchanges partition halves
    (0:32 <-> 32:64) within each 64-row block.
    """
    inv_freq = 1.0 / (ROPE_BASE ** (np.arange(0, HD, 2, dtype=np.float32) / HD))
    t = np.arange(Sz, dtype=np.float32)
    freqs = t[:, None] * inv_freq[None, :]  # [S, 32]
    emb = np.concatenate([freqs, freqs], axis=-1)  # [S, 64]
    cos = np.cos(emb).T.astype(np.float32)  # [64, S]
    sin = np.sin(emb).T.astype(np.float32)  # [64, S]
    sin2 = sin.copy()
    sin2[0:32] = -sin2[0:32]
    cosT = np.concatenate([cos, cos], axis=0)
    sinT = np.concatenate([sin2, sin2], axis=0)
    return np.ascontiguousarray(cosT), np.ascontiguousarray(sinT)


def core_inputs(x, w_qkv, w_out, core: int):
    """Host-side prep of one core's input map (bf16)."""
    import ml_dtypes

    ndt = ml_dtypes.bfloat16
    b, hg = core // 2, core % 2
    Dz = x.shape[2]
    hpc_rows = (H // 2) * HD  # 512 rows per head-group
    r0 = hg * hpc_rows
    wq = w_qkv[r0 : r0 + hpc_rows, :]
    wk = w_qkv[Dz + r0 : Dz + r0 + hpc_rows, :]
    wv_ = w_qkv[2 * Dz + r0 : 2 * Dz + r0 + hpc_rows, :]
    cosT, sinT = rope_tables(x.shape[1])
    return {
        "xt": np.ascontiguousarray(x[b].T).astype(ndt),
        "wqk": np.ascontiguousarray(np.concatenate([wq, wk], axis=0).T).astype(ndt),
        "wv": np.ascontiguousarray(wv_.T).astype(ndt),
        "wo": np.ascontiguousarray(w_out[:, r0 : r0 + hpc_rows].T).astype(ndt),
        "cost": cosT.astype(ndt),
        "sint": sinT.astype(ndt),
    }


_CACHE = {}


def kernel(x, w_qkv, w_out):
    x = np.asarray(x, dtype=np.float32)
    w_qkv = np.asarray(w_qkv, dtype=np.float32)
    w_out = np.asarray(w_out, dtype=np.float32)
    assert x.shape == (B, S, D) and w_qkv.shape == (3 * D, D) and w_out.shape == (D, D)

    from concourse.bass_utils import run_bass_kernel_spmd

    if "nc" not in _CACHE:
        _CACHE["nc"] = build_nc(Cfg())
    nc = _CACHE["nc"]

    in_maps = [core_inputs(x, w_qkv, w_out, c) for c in range(NCORES)]
    res = run_bass_kernel_spmd(nc, in_maps, core_ids=list(range(NCORES)))
    outs = [res.results[c]["out"] for c in range(NCORES)]
    full = np.empty((B, S, D), dtype=np.float32)
    for b in range(B):
        full[b] = outs[2 * b] + outs[2 * b + 1]
    return full
